# revision 5
# baseline (speedup 1.0000x reference)
"""Trainium2 Bass kernel for the k-mer transformer problem (PE-matmul version).

Semantics (k=3, one-hot 3-mer filters over 4 bases):
    z[b, c, l] = relu(x[b,0,l,d0] + x[b,0,l+1,d1] + x[b,0,l+2,d2] - 2)
      where c = 16*d0 + 4*d1 + d2,  l in [0, 99999)
    out[b, 0, r*33333 + q, c] = z[b, c, 3q + r]      (mod-3 interleave)

Strategy: pure data parallel (batch elem b -> NeuronCore b). Per core the
conv-as-matmul runs on the TensorEngine: stationary one-hot weights pack
TWO adjacent output positions (s=0,1) x 64 channels into the 128 PSUM
partitions; moving-operand column n carries the 4 x-rows (x[2n..2n+3, 0:4])
the two windows need:  psum[(s,c), n] = x[2n+s+0, d0] + x[2n+s+1, d1] +
x[2n+s+2, d2].  512 pairs per matmul (one PSUM bank), 98 matmuls covering
50176 pairs >= 50000.  Everything is fp16 (PE streams 1 row/cycle, weights
0/1 exact, x rounds at 2^-12; measured f32r was 3x slower).

The pair space is split into 8 sixteenth-row groups ("eighths") so the
input load covers all 128 SBUF partitions (a 16-partition DMA only engages
1/4 of the SDMA engines: measured 114 GB/s vs ~358).  Eighths 2q and 2q+1
share PE quad q (rows 32q..32q+32, explicit tile_position): matmuls use
K=32 where the other eighth's 16 rows carry zero weights -- K is free on
the PE.  Tiles are processed round-robin across eighths (round r = one
512-pair column block on every eighth), so a column-split load covers the
first rounds of ALL eighths and compute starts ~1 us after load #1.

Evacuation (relu(x-2) + cast to fp16) runs per 4-tile unit (2048 psum cols
= 4 banks, psum bufs=2), column-split 256/256 per tile between the scalar
engine (ACT, activation with bias=-2) and the vector engine (DVE, fused
tensor_scalar add(-2)+max(0)); each engine writes its own SBUF chunks so
every store has a single-engine dependency.  ACT chunks (6) store via the
ACT HWDGE queue (wait pre-satisfied by program order), DVE chunks (8) via
SWDGE (gpsimd), which has its own 8 sem lanes.  fp16 output halves HBM
write traffic (12.9 MB vs 25.7) at ~2^-12 rounding error, far under the
2e-2 gate; the host converts back to f32.
"""

import sys

import numpy as np

sys.path.insert(0, "/opt/trn_rl_repo")

import concourse.bacc as bacc  # noqa: E402
import concourse.mybir as mybir  # noqa: E402
from concourse.bass_utils import run_bass_kernel_spmd  # noqa: E402
from concourse.tile import TileContext  # noqa: E402

P = 128
NT = 512  # pairs per matmul = one PSUM bank of f32
TILES = 98  # 98 * 512 = 50176 pairs >= 50000 needed
ET = [13, 13, 13, 13, 12, 12, 11, 11]  # tiles per eighth
EOFF = [0, 13, 26, 39, 52, 64, 76, 87]
WCOL = 256  # two 128-col weight blocks (even/odd eighth of each quad)
XW = WCOL + 13 * NT  # staged x columns (6912)
NU = 49  # evac units of 2 tiles (2 PSUM banks; psum bufs=4 = all 8 banks)
# Unit-level engine alternation: each unit's full 1024 cols evacuated by ONE
# engine (half the instructions per engine -> PSUM-init overhead amortizes).
# ACT is faster per col (0.833 vs 1.042 ns) -> 26 ACT units vs 23 DVE units.
NA_UNITS = 26
ASSIGN = []  # True = ACT
_acc = 0
for _u in range(NU):
    _new = ((_u + 1) * NA_UNITS) // NU
    ASSIGN.append(_new > _acc)
    _acc = _new
ND_UNITS = NU - NA_UNITS
UCOL = 2 * NT  # output cols per unit (u8)
YA = NA_UNITS * UCOL
YB = ND_UNITS * UCOL
CHUNKS_A = [1, 5, 7, 8, 5]  # ACT store chunks, in A-units (sum 26)
CHUNKS_B = [1, 3, 4, 5, 5, 5]  # DVE store chunks, in D-units (sum 23)
LOAD1_COLS = WCOL + 2 * NT  # load #1 covers rounds 0-1 of every eighth
L = 100001
Q = 33333  # output rows per phase
N_CORES = 8
XF_LEN = 8 * 512 * EOFF[7] + 8 * (ET[7] * 512 - 1) + 16
# tile sequence: round-robin rounds over eighths; within a round, visit
# quads alternately (0,2,4,6,1,3,5,7) so consecutive matmuls never share a
# PE row-group -- enables the PE's LDWEIGHTS pull-ahead.
SEQ = [(e, r) for r in range(13) for e in (0, 2, 4, 6, 1, 3, 5, 7) if r < ET[e]]
assert len(SEQ) == TILES

_CACHE = {}


def _build_bass():
    nc = bacc.Bacc()
    f32 = mybir.dt.float32
    f16 = mybir.dt.float16
    u8 = mybir.dt.uint8
    sub = mybir.AluOpType.subtract
    mx = mybir.AluOpType.max
    relu = mybir.ActivationFunctionType.Relu

    x_d = nc.declare_dram_parameter("x", [P, XW], f16, isOutput=False)
    y_d = nc.declare_dram_parameter("y", [P, YA + YB], u8, isOutput=True)

    with TileContext(nc) as tc:
        with (
            tc.tile_pool(name="xp", bufs=1) as xp,
            tc.tile_pool(name="ps", bufs=4, space="PSUM") as psp,
            tc.tile_pool(name="oa", bufs=3) as oap,
            tc.tile_pool(name="ob", bufs=3) as obp,
        ):
            x_sb = xp.tile([P, XW], f16)
            nc.sync.dma_start(
                out=x_sb[:, 0:LOAD1_COLS], in_=x_d[:, 0:LOAD1_COLS]
            )
            nc.sync.dma_start(
                out=x_sb[:, LOAD1_COLS:XW], in_=x_d[:, LOAD1_COLS:XW]
            )
            bias_sb = xp.tile([P, 1], f32, tag="bias")
            nc.vector.memset(bias_sb, -510.0)

            # Per-unit evac: whole units alternate between ACT and DVE.
            a_bounds = set(np.cumsum(CHUNKS_A).tolist())
            b_bounds = set(np.cumsum(CHUNKS_B).tolist())
            s = 0  # sequence tile index
            ua = ub = 0  # A-units / D-units evacuated so far
            ua0 = ub0 = 0  # first A/D-unit of the open chunk
            oa = oap.tile([P, CHUNKS_A[0] * UCOL], u8, tag="oa")
            ob = obp.tile([P, CHUNKS_B[0] * UCOL], u8, tag="ob")
            na = nb = 0  # stores issued so far per engine
            for u in range(NU):
                ps = psp.tile([P, 2 * NT], f32, tag="ps")
                for i in range(2):
                    e, r = SEQ[s + i]
                    q, par = divmod(e, 2)
                    rows = slice(32 * q, 32 * q + 32)
                    nc.tensor.matmul(
                        ps[:, i * NT : (i + 1) * NT],
                        x_sb[rows, 128 * par : 128 * par + 128],
                        x_sb[rows, WCOL + r * NT : WCOL + (r + 1) * NT],
                        start=True,
                        stop=True,
                        tile_position=(32 * q, 0),
                    )
                s += 2
                if ASSIGN[u]:
                    ka = (ua - ua0) * UCOL
                    nc.scalar.activation(
                        oa[:, ka : ka + UCOL], ps[:], relu, bias=bias_sb
                    )
                    ua += 1
                    if ua in a_bounds:
                        nc.sync.dma_start(
                            out=y_d[:, ua0 * UCOL : ua * UCOL], in_=oa
                        )
                        na += 1
                        ua0 = ua
                        if na < len(CHUNKS_A):
                            oa = oap.tile(
                                [P, CHUNKS_A[na] * UCOL], u8, tag="oa"
                            )
                else:
                    kb = (ub - ub0) * UCOL
                    nc.vector.tensor_scalar(
                        out=ob[:, kb : kb + UCOL],
                        in0=ps[:],
                        scalar1=510.0,
                        scalar2=510.0,
                        op0=mx,
                        op1=sub,
                    )
                    ub += 1
                    if ub in b_bounds:
                        nc.gpsimd.dma_start(
                            out=y_d[:, YA + ub0 * UCOL : YA + ub * UCOL],
                            in_=ob,
                        )
                        nb += 1
                        ub0 = ub
                        if nb < len(CHUNKS_B):
                            ob = obp.tile(
                                [P, CHUNKS_B[nb] * UCOL], u8, tag="ob"
                            )
    return nc


def _weights():
    w = np.zeros((16, 128), dtype=np.float16)
    for c in range(64):
        digs = (c >> 4 & 3, c >> 2 & 3, c & 3)
        for s in range(2):
            for j, d in enumerate(digs):
                w[4 * (s + j) + d, s * 64 + c] += 255.0
    return w


def _stage_inputs(x):
    """x: [8, 1, L, 4] f32 -> list of per-core {'x': [8, 16, XW] f16}."""
    w = _weights()
    in_maps = []
    for b in range(x.shape[0]):
        xf = np.zeros(XF_LEN, dtype=np.float16)
        xf[: L * 4] = x[b, 0].ravel()
        xs = np.zeros((8, 16, XW), dtype=np.float16)
        for e in range(8):
            xs[e, :, 128 * (e % 2) : 128 * (e % 2) + 128] = w
            ncols = ET[e] * 512
            v = np.lib.stride_tricks.as_strided(
                xf[8 * 512 * EOFF[e] :],
                shape=(4, 4, ncols),
                strides=(8, 2, 16),
            )
            xs[e, :, WCOL : WCOL + ncols] = v.reshape(16, ncols)
        in_maps.append({"x": xs.reshape(P, XW)})
    return in_maps


# seq index s -> global tile EOFF[e]+r; inv[g] = s
_G_OF_S = np.array([EOFF[e] + r for (e, r) in SEQ])
_S_OF_G = np.argsort(_G_OF_S)


def _gather_output(results):
    # y block for SEQ tile sq lives in unit u = sq//2 at half sq%2; units are
    # split between region A (ACT, in A-unit order) and B (DVE, in D-unit
    # order) after YA.
    src = np.empty(TILES, dtype=np.int64)
    ua = ub = 0
    for u in range(NU):
        if ASSIGN[u]:
            base = ua * UCOL
            ua += 1
        else:
            base = YA + ub * UCOL
            ub += 1
        src[2 * u] = base
        src[2 * u + 1] = base + NT
    out = np.empty((len(results), 1, 3 * Q, 64), dtype=np.float32)
    cols = (src[:, None] + np.arange(NT)[None, :]).ravel()  # seq-tile order
    for b, res in enumerate(results):
        y = np.asarray(res["y"])  # [128, YA+YB] u8
        yf = y[:, cols].reshape(P, TILES, NT)[:, _S_OF_G]  # global order
        z = (
            yf.reshape(2, 64, TILES * NT)
            .transpose(2, 0, 1)
            .reshape(2 * TILES * NT, 64)[: 3 * Q]
        )  # [pos, c] conv order
        out[b, 0] = z.reshape(Q, 3, 64).transpose(1, 0, 2).reshape(
            3 * Q, 64
        ).astype(np.float32) * np.float32(1.0 / 255.0)
    return out


def _built_and_finalized():
    if "nc" not in _CACHE:
        nc = _build_bass()
        nc.finalize()
        _CACHE["nc"] = nc
    return _CACHE["nc"]


def run(x, trace=False):
    nc = _built_and_finalized()
    in_maps = _stage_inputs(np.asarray(x, dtype=np.float32))
    bkr = run_bass_kernel_spmd(nc, in_maps, list(range(N_CORES)), trace=trace)
    return _gather_output(bkr.results), bkr


def kernel(x, W=None):
    out, _ = run(x, trace=False)
    return out
